# revision 47
# baseline (speedup 1.0000x reference)
"""Trainium2 Bass kernel for nn_AttnLayerV3 (differential attention layer).

v2 — tensor-parallel over (kv-head x batch) across 8 NeuronCores:
  - core c owns batch b = c // 4 and kv-head g = c % 4 (4 q-heads, 1024 tokens)
  - no duplicated K/V projection work (vs v1's core-pair duplication)
  - ACT engine runs exp ONLY (all PSUM drains on DVE; GroupNorm rstd via
    fast-inverse-sqrt Newton on DVE) -> no activation-table thrash
  - software-pipelined emission: scores matmuls interleaved with projection /
    o-proj matmuls so the PE never stalls on exp and HAM stays warm
  - host: shards weights, gathers 8 partial outputs (4 per batch) and sums.

All matmuls in bf16 with fp32 PSUM accumulation.
"""

import contextlib

import numpy as np
import ml_dtypes

import concourse.bass as bass
import concourse.bacc as bacc
import concourse.tile as tile
import concourse.mybir as mybir
from concourse.bass_utils import run_bass_kernel_spmd
from concourse.masks import make_identity, make_upper_triangular

bf16 = ml_dtypes.bfloat16

B, T, D = 2, 1024, 2048
H, KV, DH = 16, 4, 128
NCORES = 8
HPC = 4                    # q heads per core
LAMBDA_INIT = 0.8 - 0.6 * float(np.exp(-0.3 * 1))
GN_EPS = 1e-5
ROPE_BASE = 10000.0

KB = 16                    # contraction blocks of 128 over D
NEWTON_ITERS = 1           # rsqrt Newton refinements (1 -> ~2e-3 rel)
NTB = T // 128             # 8 token blocks of 128 per core
NJ = T // 512              # 2 query superblocks of 512
MAGICF = float(np.frombuffer(np.uint32(0x5F3759DF).tobytes(), np.float32)[0])


def build_program(lam: float):
    f32 = mybir.dt.float32
    bf = mybir.dt.bfloat16
    nc = bacc.Bacc("TRN2", target_bir_lowering=False, debug=False,
                   num_devices=NCORES)

    xT_d = nc.dram_tensor("xT", (NJ, 128, KB, 512), bf, kind="ExternalInput").ap()
    wq_d = nc.dram_tensor("wqT", (8, 128, KB, 128), bf, kind="ExternalInput").ap()
    wk_d = nc.dram_tensor("wkT", (2, 128, KB, 128), bf, kind="ExternalInput").ap()
    wv_d = nc.dram_tensor("wvT", (128, KB, 256), bf, kind="ExternalInput").ap()
    wo_d = nc.dram_tensor("woT", (128, 8, D), bf, kind="ExternalInput").ap()
    tbl_d = nc.dram_tensor("tbl", (128, T), bf, kind="ExternalInput").ap()
    tb2_d = nc.dram_tensor("tbl2", (128, T), bf, kind="ExternalInput").ap()
    out_d = nc.dram_tensor("out", (T, D), bf, kind="ExternalOutput").ap()

    with tile.TileContext(nc) as tc:
        _body(tc, xT_d, wq_d, wk_d, wv_d, wo_d, tbl_d, tb2_d, out_d, lam)
    nc.compile()
    return nc


def _body(tc, xT_d, wq_d, wk_d, wv_d, wo_d, tbl_d, tb2_d, out_d, lam):
    nc = tc.nc
    f32 = mybir.dt.float32
    i32 = mybir.dt.int32
    bf = mybir.dt.bfloat16
    mult = mybir.AluOpType.mult
    sub = mybir.AluOpType.subtract
    add = mybir.AluOpType.add
    shr = mybir.AluOpType.arith_shift_right

    ctx = contextlib.ExitStack()
    with ctx:
        # ---- pools -------------------------------------------------------
        big = ctx.enter_context(tc.tile_pool(name="big", bufs=1))
        wpool = ctx.enter_context(tc.tile_pool(name="w", bufs=4))
        rtmp = ctx.enter_context(tc.tile_pool(name="rtmp", bufs=2))
        epool = ctx.enter_context(tc.tile_pool(name="e", bufs=2))
        opool = ctx.enter_context(tc.tile_pool(name="o", bufs=6))
        spool = ctx.enter_context(tc.tile_pool(name="s", bufs=5))
        stage = ctx.enter_context(tc.tile_pool(name="stage", bufs=2))
        pmm = ctx.enter_context(tc.tile_pool(name="pmm", bufs=2, space="PSUM"))
        pav = ctx.enter_context(tc.tile_pool(name="pav", bufs=2, space="PSUM"))
        pop = ctx.enter_context(tc.tile_pool(name="pop", bufs=2, space="PSUM"))
        # PSUM banks: pmm 2 + pav (u1:2 + u2:2) 4 + pop 2 = 8

        # ---- constants ---------------------------------------------------
        ident = big.tile([128, 128], bf, tag="ident")
        make_identity(nc, ident[:])
        tri = big.tile([128, 128], bf, tag="tri")
        make_upper_triangular(nc, tri[:], val=1.0, diag=True)
        magic = big.tile([128, 4], f32, tag="magic")
        nc.vector.memset(magic[:], MAGICF)

        # ---- resident tensors & input DMAs ------------------------------
        wts = {}

        def dma_w(idx, src):
            wt = wpool.tile([128, KB, 128], bf, tag="w", name=f"wt{idx}")
            nc.sync.dma_start(wt[:], src)
            wts[idx] = wt

        # k streams first (unblock first matmuls); q blocks follow.
        # First transfers issue from four different engine queues so the
        # ~1us per-issue cost overlaps.
        xsb = big.tile([128, KB, T], bf, tag="xsb")
        wt_k0 = wpool.tile([128, KB, 128], bf, tag="w", name="wtk0")
        nc.scalar.dma_start(wt_k0[:], wk_d[0])
        wts["k0"] = wt_k0
        nc.sync.dma_start(xsb[:, 0:4, 0:512], xT_d[0][:, 0:4])
        nc.scalar.dma_start(xsb[:, 4:8, 0:512], xT_d[0][:, 4:8])
        nc.gpsimd.dma_start(xsb[:, 8:12, 0:512], xT_d[0][:, 8:12])
        nc.sync.dma_start(xsb[:, 12:16, 0:512], xT_d[0][:, 12:16])
        dma_w("k1", wk_d[1])
        # keep the PE busy (and the HAM clock-gate open) through the DMA
        # lead-in with throwaway transposes of the identity
        warm = pop.tile([128, 512], bf, tag="op", name="warm")
        for _ in range(70):
            nc.tensor.matmul(warm[:, 0:128], ident[:], ident[:],
                             start=True, stop=True, is_transpose=True,
                             skip_group_check=True)
        tbl = big.tile([128, T], bf, tag="tbl")        # cos rows 0:64, sin 64:128
        nc.sync.dma_start(tbl[:], tbl_d)
        tbl2 = big.tile([128, T], bf, tag="tbl2")      # sin rows 0:64, cos 64:128
        nc.sync.dma_start(tbl2[:], tb2_d)
        dma_w("q0", wq_d[0])
        wv = big.tile([128, KB, 256], bf, tag="wv")
        nc.sync.dma_start(wv[:], wv_d)
        dma_w("q1", wq_d[1])
        nc.sync.dma_start(xsb[:, :, 512:1024], xT_d[1])
        wo_sb = big.tile([128, 8, D], bf, tag="wo")

        tblC = tbl      # [cos; cos] both halves
        tblS = tbl2     # [-sin; sin]

        # qk: 8 q blocks (2h+s) then 2 k blocks (8+s); roped, dh-permuted
        qk = big.tile([128, 10, T], bf, tag="qk")
        vsb = big.tile([128, NTB, 257], bf, tag="vsb")
        nc.vector.memset(vsb[:, :, 256:257], 1.0)
        osbT = big.tile([128, 8, T], bf, tag="osbT")   # o transposed, ch-major

        # ---- emission helpers -------------------------------------------
        def proj_group(widx, blk, ts):
            """Q/K projection of one 128-row block for one 512-token span,
            PSUM drained by DVE cast, rope applied by DVE into qk[:, blk]."""
            wt = wts[widx]
            tsl = slice(ts * 512, (ts + 1) * 512)
            ps = pmm.tile([128, 512], f32, tag="mm", name="psp")
            for k in range(KB):
                nc.tensor.matmul(ps[:], wt[:, k], xsb[:, k, tsl],
                                 start=(k == 0), stop=(k == KB - 1))
            # full-width rope: qk = raw*[cos;cos64] + swap(raw)*[-sin;sin0]
            raw = rtmp.tile([128, 512], bf, tag="raw")
            nc.vector.tensor_copy(raw[:], ps[:])
            sw = rtmp.tile([128, 512], bf, tag="sw")
            nc.vector.tensor_copy(sw[0:64, :], raw[64:128, :])
            nc.vector.tensor_copy(sw[64:128, :], raw[0:64, :])
            t1 = rtmp.tile([128, 512], bf, tag="t1")
            t2 = rtmp.tile([128, 512], bf, tag="t2")
            nc.vector.tensor_tensor(t1[:], sw[:], tblS[:, tsl], mult)
            nc.vector.tensor_tensor(t2[:], raw[:], tblC[:, tsl], mult)
            nc.vector.tensor_tensor(qk[:, blk, tsl], t2[:], t1[:], add)

        def v_group(tb):
            ps = pmm.tile([128, 512], f32, tag="mm", name="psv")
            for k in range(KB):
                nc.tensor.matmul(ps[:, 0:256], xsb[:, k, tb * 128:(tb + 1) * 128],
                                 wv[:, k], start=(k == 0), stop=(k == KB - 1))
            nc.vector.tensor_copy(vsb[:, tb, 0:256], ps[:, 0:256])

        def scores_block(h, s, i, J, e_t):
            """One k-block of scores for (head h, stream s, superblock J):
            matmul + exp (ACT) + causal mask for the diagonal block."""
            il = i - 4 * J
            off = il * 128 if il >= 0 else 0
            fd = 512 - off
            ps = pmm.tile([128, 512], f32, tag="mm", name="pssc")
            nc.tensor.matmul(ps[:, 0:fd], qk[:, 8 + s, i * 128:(i + 1) * 128],
                             qk[:, 2 * h + s, J * 512 + off:(J + 1) * 512],
                             start=True, stop=True)
            nc.scalar.activation(e_t[:, i, off:512], ps[:, 0:fd],
                                 mybir.ActivationFunctionType.Exp)
            if il >= 0:
                eng = nc.gpsimd if J == 0 else nc.vector
                eng.tensor_tensor(e_t[:, i, off:off + 128],
                                  e_t[:, i, off:off + 128], tri[:], mult)

        def rsqrt_newton(vb):
            """rstd = 1/sqrt(vb) elementwise on [128,4] via DVE only."""
            y = spool.tile([128, 4], f32, tag="ny")
            t = spool.tile([128, 4], f32, tag="nt")
            nc.vector.tensor_scalar(y[:].bitcast(i32), vb[:].bitcast(i32),
                                    1, None, shr)
            nc.vector.tensor_tensor(y[:].bitcast(i32), magic[:].bitcast(i32),
                                    y[:].bitcast(i32), sub)
            for _ in range(2):
                nc.vector.tensor_tensor(t[:], y[:], y[:], mult)
                nc.vector.tensor_tensor(t[:], t[:], vb[:], mult)
                nc.vector.tensor_scalar(t[:], t[:], -0.5, 1.5, mult, add)
                nc.vector.tensor_tensor(y[:], y[:], t[:], mult)
                if NEWTON_ITERS == 1:
                    break
            return y

        def av_jl(st, jl):
            """AV matmuls + stream combine + accumulated GN stats for one
            128-token query block; frees its PSUM immediately."""
            h, J, e1, e2 = st["h"], st["J"], st["e1"], st["e2"]
            jg = 4 * J + jl
            qq = slice(jl * 128, (jl + 1) * 128)
            u1f = pav.tile([128, 512], f32, tag="u1", name="u1")
            u2f = pav.tile([128, 512], f32, tag="u2", name="u2")
            u1 = u1f[:, 0:257]
            u2 = u2f[:, 0:257]
            for i in range(jg + 1):
                vt = vsb[:, i, :]
                nc.tensor.matmul(u1, e1[:, i, qq], vt,
                                 start=(i == 0), stop=(i == jg),
                                 skip_group_check=True)
                nc.tensor.matmul(u2, e2[:, i, qq], vt,
                                 start=(i == 0), stop=(i == jg),
                                 skip_group_check=True)
            r1 = spool.tile([128, 1], f32, tag="r1")
            r2 = spool.tile([128, 1], f32, tag="r2")
            nc.vector.reciprocal(r1[:], u1[:, 256:257])
            nc.vector.reciprocal(r2[:], u2[:, 256:257])
            nc.vector.tensor_scalar_mul(r2[:], r2[:], -lam)
            o1 = opool.tile([128, 256], bf, tag="o1", name=f"o1_{jl}")
            nc.vector.tensor_scalar_mul(o1[:], u1f[:, 0:256], r1[:])
            nc.vector.scalar_tensor_tensor(o1[:], u2f[:, 0:256], r2[:],
                                           o1[:], mult, add)
            st6 = spool.tile([128, 6], f32, tag="st6")
            mv = spool.tile([128, 2], f32, tag="mv", name=f"mv_{jl}")
            nc.vector.bn_stats(st6[:], o1[:])
            nc.vector.bn_aggr(mv[:], st6[:])
            st["o1s"].append(o1)
            st["mvs"].append(mv)

        def av_fin(st, fillers=()):
            """GroupNorm normalize + transposes into osbT. `fillers` are
            tensor-work closures emitted while the DVE chain completes."""
            h, J = st["h"], st["J"]
            vb = spool.tile([128, 4], f32, tag="vb", name="vb")
            for jl in range(4):
                nc.vector.tensor_scalar_add(vb[:, jl:jl + 1],
                                            st["mvs"][jl][:, 1:2], GN_EPS)
            rst = rsqrt_newton(vb)
            o2s = []
            for jl in range(4):
                o2 = opool.tile([128, 256], bf, tag="o2", name=f"o2_{jl}")
                nc.vector.tensor_scalar(o2[:], st["o1s"][jl][:],
                                        st["mvs"][jl][:, 0:1],
                                        rst[:, jl:jl + 1], sub, mult)
                o2s.append(o2)
            for f in fillers:
                f()
            # transposes (q,ch)->(ch,q) into resident osbT, via bf16 psum
            pst1 = pav.tile([128, 512], bf, tag="u1", name="pst1")
            pst2 = pav.tile([128, 512], bf, tag="u2", name="pst2")
            for jl in range(4):
                cc = slice(jl * 128, (jl + 1) * 128)
                nc.tensor.matmul(pst1[:, cc], o2s[jl][:, 0:128], ident[:],
                                 start=True, stop=True, is_transpose=True,
                                 skip_group_check=True)
                nc.tensor.matmul(pst2[:, cc], o2s[jl][:, 128:256], ident[:],
                                 start=True, stop=True, is_transpose=True,
                                 skip_group_check=True)
            jsl = slice(J * 512, (J + 1) * 512)
            nc.vector.tensor_copy(osbT[:, 2 * h, jsl], pst1[:])
            nc.vector.tensor_copy(osbT[:, 2 * h + 1, jsl], pst2[:])

        def av_state(h, J, e1, e2):
            return {"h": h, "J": J, "e1": e1, "e2": e2,
                    "o1s": [], "mvs": []}

        # o-projection: per token block tb, 4 psum chunks of 512 out-cols,
        # each an 8-step accumulation; emitted as 4-mm half-chunks.
        # lo=0 halves touch only cb 0..3 (heads 0,1); lo=4 touch cb 4..7.
        ostage = {}
        odrained = {}

        def oproj_half(tb, n, lo, split_dma=False):
            if tb not in ostage:
                ostage[tb] = stage.tile([128, D], bf, tag="so",
                                        name=f"so{tb}")
                odrained[tb] = 0
            if lo == 0:
                ostage[(tb, n)] = pop.tile([128, 512], f32, tag="op",
                                           name=f"pso{tb}_{n}")
            pso = ostage[(tb, n)]
            for cb in range(lo, lo + 4):
                nc.tensor.matmul(pso[:], osbT[:, cb, tb * 128:(tb + 1) * 128],
                                 wo_sb[:, cb, n * 512:(n + 1) * 512],
                                 start=(cb == 0), stop=(cb == 7),
                                 skip_group_check=True)
            if lo == 4:
                nc.vector.tensor_copy(ostage[tb][:, n * 512:(n + 1) * 512],
                                      pso[:])
                del ostage[(tb, n)]
                odrained[tb] += 1
                if split_dma:
                    eng = nc.scalar if tb % 2 else nc.sync
                    eng.dma_start(
                        out_d[tb * 128:(tb + 1) * 128, n * 512:(n + 1) * 512],
                        ostage[tb][:, n * 512:(n + 1) * 512])
                    if odrained[tb] == 4:
                        del ostage[tb]
                elif odrained[tb] == 4:
                    nc.sync.dma_start(out_d[tb * 128:(tb + 1) * 128, :],
                                      ostage[tb][:])
                    del ostage[tb]

        # ---- block AB: projections + J0 attention interleaved ------------
        proj_group("k0", 8, 0)
        proj_group("k1", 9, 0)
        proj_group("q0", 0, 0)
        proj_group("q1", 1, 0)
        v_group(0)
        proj_group("k0", 8, 1)
        proj_group("k1", 9, 1)
        dma_w("q2", wq_d[2])
        dma_w("q3", wq_d[3])
        proj_group("q0", 0, 1)
        proj_group("q1", 1, 1)
        dma_w("q4", wq_d[4])
        dma_w("q5", wq_d[5])
        for half in range(2):
            nc.sync.dma_start(wo_sb[:, 4 * half:4 * half + 4],
                              wo_d[:, 4 * half:4 * half + 4])
        for tb in range(1, 4):
            v_group(tb)

        # per-h filler slots (one after each of the 8 J0 scores blocks);
        # av-jl chunks of the previous head fill the remaining slots
        fillers = {
            0: [("q2", 2, 0), ("q2", 2, 1), ("q3", 3, 0), ("q3", 3, 1),
                ("q4", 4, 0), ("q4", 4, 1), ("q5", 5, 0), ("q5", 5, 1)],
            1: [("q6", 6, 0), ("q6", 6, 1), ("q7", 7, 0), ("q7", 7, 1)],
            2: [("v", 4), ("v", 5)],
            3: [("v", 6), ("v", 7)],
        }
        dma_after = {0: ["q6", "q7"], 1: [], 2: [], 3: []}

        def emit_filler(f):
            if f[0] == "v":
                v_group(f[1])
            else:
                proj_group(*f)

        prev_st = None
        for h in range(HPC):
            e1 = epool.tile([128, 8, 512], bf, tag="e1", name=f"e1_{h}j0")
            e2 = epool.tile([128, 8, 512], bf, tag="e2", name=f"e2_{h}j0")
            st = av_state(h, 0, e1, e2)
            blocks = [(s, i) for s in range(2) for i in range(4)]
            fl = list(fillers[h])
            # hold the last filler back: it runs inside av_fin while the
            # GroupNorm DVE chain completes, so the transposes don't stall
            # the tensor queue (the held item must not gate J0 scores)
            fin_fill = fl.pop() if (prev_st is not None and fl) else None
            for bi, (s, i) in enumerate(blocks):
                scores_block(h, s, i, 0, e1 if s == 0 else e2)
                # av-jl chunks of the previous head early (frees PSUM, puts
                # the combine ahead of the rope work on the DVE queue)
                if prev_st is not None and bi % 2 == 0:
                    av_jl(prev_st, bi // 2)
                elif fl:
                    f = fl.pop(0)
                    if f is not None:
                        emit_filler(f)
                if prev_st is not None and fl and bi % 2 == 1:
                    f = fl.pop(0)
                    if f is not None:
                        emit_filler(f)
            for widx in dma_after[h]:
                dma_w(widx, wq_d[int(widx[1:])])
            if prev_st is not None:
                ff = fin_fill
                av_fin(prev_st,
                       fillers=[] if ff is None
                       else [lambda: emit_filler(ff)])
            prev_st = st

        # ---- block C: J1 attention + o-projection interleaved ------------
        # J0 o-proj halves: lo=0 (cb 0..3) are legal once heads 0,1 J0 are
        # transposed; lo=4 (cb 4..7) only after the J0 pipeline drains (end
        # of h=0 iteration here).
        opq = [(tb, n, lo) for tb in range(4) for n in range(4)
               for lo in (0, 4)]
        reserve = [opq.pop() for _ in range(6)][::-1]   # tail halves for av_fin(3,J1)

        for h in range(HPC):
            e1 = epool.tile([128, 8, 512], bf, tag="e1", name=f"e1_{h}j1")
            e2 = epool.tile([128, 8, 512], bf, tag="e2", name=f"e2_{h}j1")
            st = av_state(h, 1, e1, e2)
            blocks = [(s, i) for s in range(2) for i in range(8)]
            ps_prev = prev_st
            spacers = [lambda jl=jl: av_jl(ps_prev, jl) for jl in range(4)]
            if h == 0:
                # prev = (3, J0): lo=0 halves (cb 0..3) are legal now; lo=4
                # (cb 4..7) only after its av_fin lands the J0 transposes,
                # so those go in the post-scores tail
                pre = [opq[0], opq[2]]                  # (t0n0lo0) (t0n1lo0)
                del opq[2], opq[0]
                tail, opq[:3] = opq[:3], []             # lo4s + next lo0
                spacers += [lambda a=a: oproj_half(*a) for a in pre]
                spacers.append(lambda: av_fin(ps_prev))
                spacers += [lambda a=a: oproj_half(*a) for a in tail]
            else:
                nh = 4 if h < 3 else 2
                pre, opq[:nh] = opq[:nh], []
                post, opq[:2] = opq[:2], []
                spacers += [lambda a=a: oproj_half(*a) for a in pre]
                spacers.append(lambda: av_fin(
                    ps_prev,
                    fillers=[lambda a=a: oproj_half(*a) for a in post]))
            for bi, (s, i) in enumerate(blocks):
                scores_block(h, s, i, 1, e1 if s == 0 else e2)
                if bi % 2 == 1 and spacers:
                    f = spacers.pop(0)
                    f()
            while spacers:
                spacers.pop(0)()
            prev_st = st

        res_ops = [lambda a=a: oproj_half(*a) for a in reserve]
        for jl in range(4):
            av_jl(prev_st, jl)
            res_ops.pop(0)()
        av_fin(prev_st, fillers=res_ops)

        # ---- block D: J1 o-projection ------------------------------------
        for a in opq:
            oproj_half(*a)
        for tb in range(4, 8):
            for n in range(4):
                oproj_half(tb, n, 0)
                oproj_half(tb, n, 4, split_dma=(tb >= 6))


# ------------------------- host side  ------------------------------------

_ROPE_PERM = np.concatenate([np.arange(0, DH, 2), np.arange(1, DH, 2)])


def _prep(x, Wq, Wk, Wv, Wo, lambda_q1, lambda_k1, lambda_q2, lambda_k2,
          gn_weight, gn_bias, pos):
    lam = float(np.exp(np.sum(lambda_q1 * lambda_k1))
                - np.exp(np.sum(lambda_q2 * lambda_k2)) + LAMBDA_INIT)
    scale = DH ** -0.5

    posf = pos.astype(np.float64)
    inv = 1.0 / (ROPE_BASE ** (np.arange(0, DH, 2, dtype=np.float32) / DH))
    freqs = (posf[:, None] * inv[None, :].astype(np.float64)).astype(np.float32)
    cosv = np.cos(freqs).T          # (64, T)
    sinv = np.sin(freqs).T
    tbl = np.ascontiguousarray(
        np.concatenate([cosv, cosv], axis=0).astype(bf16))      # (128, T)
    tbl2 = np.ascontiguousarray(
        np.concatenate([-sinv, sinv], axis=0).astype(bf16))     # (128, T)

    # Wq: (H,2,DH,D), rope-permute DH, fold score scale
    Wq4 = (Wq.reshape(H, 2, DH, D)[:, :, _ROPE_PERM, :] * scale).astype(np.float32)
    Wk4 = Wk.reshape(KV, 2, DH, D)[:, :, _ROPE_PERM, :].astype(np.float32)
    Wv3 = Wv.reshape(KV, 2 * DH, D).astype(np.float32)

    s1 = 1.0 - LAMBDA_INIT
    Wo_f = (Wo * (gn_weight * s1)[None, :]).astype(np.float32)   # (D, 4096)
    bias_out = (gn_bias * s1).astype(np.float32) @ Wo.T.astype(np.float32)

    def to_sb(w2d, cols):           # (D, cols) -> (128, KB, cols) bf16
        return np.ascontiguousarray(
            w2d.reshape(KB, 128, cols).transpose(1, 0, 2)).astype(bf16)

    in_maps = []
    for c in range(NCORES):
        b, g = c // 4, c % 4
        # x for batch b: (T, D) -> (D, T) -> (NJ, 128, KB, 512)
        xT = x[b].T.astype(np.float32)
        x3 = xT.reshape(KB, 128, NJ, 512).transpose(2, 1, 0, 3)
        xT3 = np.ascontiguousarray(x3).astype(bf16)

        # q heads 4g..4g+3: block 2hh+s = Wq4[4g+hh, s] (DH rows)
        wqT = Wq4[4 * g:4 * g + 4].reshape(8 * DH, D).T       # (D, 1024)
        wkT = Wk4[g].reshape(2 * DH, D).T                     # (D, 256)
        wvT = Wv3[g].T                                        # (D, 256)
        woT = Wo_f[:, 1024 * g:1024 * (g + 1)].T              # (1024 ch, D)
        wo3 = np.ascontiguousarray(
            woT.reshape(8, 128, D).transpose(1, 0, 2)).astype(bf16)
        wq_stack = np.stack([to_sb(wqT[:, cb * 128:(cb + 1) * 128], 128)
                             for cb in range(8)])             # (8,128,KB,128)
        wk_stack = np.stack([to_sb(wkT[:, cb * 128:(cb + 1) * 128], 128)
                             for cb in range(2)])             # (2,128,KB,128)
        in_maps.append({
            "xT": xT3,
            "wqT": wq_stack,
            "wkT": wk_stack,
            "wvT": to_sb(wvT, 256),
            "woT": wo3,
            "tbl": tbl,
            "tbl2": tbl2,
        })
    return lam, in_maps, bias_out


LAST_RESULT = None


def kernel(**inputs):
    global LAST_RESULT
    inputs = {k: np.asarray(v) for k, v in inputs.items()}
    lam, in_maps, bias_out = _prep(**inputs)
    nc = build_program(lam)
    res = run_bass_kernel_spmd(nc, in_maps, core_ids=list(range(NCORES)))
    LAST_RESULT = res
    out = np.zeros((B, T, D), np.float32)
    for c in range(NCORES):
        out[c // 4] += res.results[c]["out"].astype(np.float32)
    out += bias_out[None, None, :]
    return out.astype(np.float32)


if __name__ == "__main__":
    import reference
    ins = {k: np.asarray(v) for k, v in reference.setup_inputs().items()}
    got = kernel(**ins)
    exp = np.asarray(reference.reference(**ins))
    rel = np.linalg.norm(got - exp) / np.linalg.norm(exp)
    print("rel err:", rel)


# revision 48
# speedup vs baseline: 1.0112x; 1.0112x over previous
"""Trainium2 Bass kernel for nn_AttnLayerV3 (differential attention layer).

v2 — tensor-parallel over (kv-head x batch) across 8 NeuronCores:
  - core c owns batch b = c // 4 and kv-head g = c % 4 (4 q-heads, 1024 tokens)
  - no duplicated K/V projection work (vs v1's core-pair duplication)
  - ACT engine runs exp ONLY (all PSUM drains on DVE; GroupNorm rstd via
    fast-inverse-sqrt Newton on DVE) -> no activation-table thrash
  - software-pipelined emission: scores matmuls interleaved with projection /
    o-proj matmuls so the PE never stalls on exp and HAM stays warm
  - host: shards weights, gathers 8 partial outputs (4 per batch) and sums.

All matmuls in bf16 with fp32 PSUM accumulation.
"""

import contextlib

import numpy as np
import ml_dtypes

import concourse.bass as bass
import concourse.bacc as bacc
import concourse.tile as tile
import concourse.mybir as mybir
from concourse.bass_utils import run_bass_kernel_spmd
from concourse.masks import make_identity, make_upper_triangular

bf16 = ml_dtypes.bfloat16

B, T, D = 2, 1024, 2048
H, KV, DH = 16, 4, 128
NCORES = 8
HPC = 4                    # q heads per core
LAMBDA_INIT = 0.8 - 0.6 * float(np.exp(-0.3 * 1))
GN_EPS = 1e-5
ROPE_BASE = 10000.0

KB = 16                    # contraction blocks of 128 over D
NEWTON_ITERS = 1           # rsqrt Newton refinements (1 -> ~2e-3 rel)
NTB = T // 128             # 8 token blocks of 128 per core
NJ = T // 512              # 2 query superblocks of 512
MAGICF = float(np.frombuffer(np.uint32(0x5F3759DF).tobytes(), np.float32)[0])


def build_program(lam: float):
    f32 = mybir.dt.float32
    bf = mybir.dt.bfloat16
    nc = bacc.Bacc("TRN2", target_bir_lowering=False, debug=False,
                   num_devices=NCORES)

    xT_d = nc.dram_tensor("xT", (NJ, 128, KB, 512), bf, kind="ExternalInput").ap()
    wq_d = nc.dram_tensor("wqT", (8, 128, KB, 128), bf, kind="ExternalInput").ap()
    wk_d = nc.dram_tensor("wkT", (2, 128, KB, 128), bf, kind="ExternalInput").ap()
    wv_d = nc.dram_tensor("wvT", (128, KB, 256), bf, kind="ExternalInput").ap()
    wo_d = nc.dram_tensor("woT", (128, 8, D), bf, kind="ExternalInput").ap()
    tbl_d = nc.dram_tensor("tbl", (128, T), bf, kind="ExternalInput").ap()
    tb2_d = nc.dram_tensor("tbl2", (128, T), bf, kind="ExternalInput").ap()
    out_d = nc.dram_tensor("out", (T, D), bf, kind="ExternalOutput").ap()

    with tile.TileContext(nc) as tc:
        _body(tc, xT_d, wq_d, wk_d, wv_d, wo_d, tbl_d, tb2_d, out_d, lam)
    nc.compile()
    return nc


def _body(tc, xT_d, wq_d, wk_d, wv_d, wo_d, tbl_d, tb2_d, out_d, lam):
    nc = tc.nc
    f32 = mybir.dt.float32
    i32 = mybir.dt.int32
    bf = mybir.dt.bfloat16
    mult = mybir.AluOpType.mult
    sub = mybir.AluOpType.subtract
    add = mybir.AluOpType.add
    shr = mybir.AluOpType.arith_shift_right

    ctx = contextlib.ExitStack()
    with ctx:
        # ---- pools -------------------------------------------------------
        big = ctx.enter_context(tc.tile_pool(name="big", bufs=1))
        wpool = ctx.enter_context(tc.tile_pool(name="w", bufs=4))
        rtmp = ctx.enter_context(tc.tile_pool(name="rtmp", bufs=2))
        epool = ctx.enter_context(tc.tile_pool(name="e", bufs=2))
        opool = ctx.enter_context(tc.tile_pool(name="o", bufs=6))
        spool = ctx.enter_context(tc.tile_pool(name="s", bufs=5))
        stage = ctx.enter_context(tc.tile_pool(name="stage", bufs=2))
        pmm = ctx.enter_context(tc.tile_pool(name="pmm", bufs=2, space="PSUM"))
        pav = ctx.enter_context(tc.tile_pool(name="pav", bufs=2, space="PSUM"))
        pop = ctx.enter_context(tc.tile_pool(name="pop", bufs=2, space="PSUM"))
        # PSUM banks: pmm 2 + pav (u1:2 + u2:2) 4 + pop 2 = 8

        # ---- constants ---------------------------------------------------
        ident = big.tile([128, 128], bf, tag="ident")
        make_identity(nc, ident[:])
        tri = big.tile([128, 128], bf, tag="tri")
        make_upper_triangular(nc, tri[:], val=1.0, diag=True)
        magic = big.tile([128, 4], f32, tag="magic")
        nc.vector.memset(magic[:], MAGICF)

        # ---- resident tensors & input DMAs ------------------------------
        wts = {}

        def dma_w(idx, src):
            wt = wpool.tile([128, KB, 128], bf, tag="w", name=f"wt{idx}")
            nc.sync.dma_start(wt[:], src)
            wts[idx] = wt

        # k streams first (unblock first matmuls); q blocks follow.
        # First transfers issue from four different engine queues so the
        # ~1us per-issue cost overlaps.
        xsb = big.tile([128, KB, T], bf, tag="xsb")
        wt_k0 = wpool.tile([128, KB, 128], bf, tag="w", name="wtk0")
        nc.scalar.dma_start(wt_k0[:], wk_d[0])
        wts["k0"] = wt_k0
        nc.sync.dma_start(xsb[:, 0:4, 0:512], xT_d[0][:, 0:4])
        nc.scalar.dma_start(xsb[:, 4:8, 0:512], xT_d[0][:, 4:8])
        nc.gpsimd.dma_start(xsb[:, 8:12, 0:512], xT_d[0][:, 8:12])
        nc.sync.dma_start(xsb[:, 12:16, 0:512], xT_d[0][:, 12:16])
        dma_w("k1", wk_d[1])
        # keep the PE busy (and the HAM clock-gate open) through the DMA
        # lead-in with throwaway transposes of the identity
        warm = pop.tile([128, 512], bf, tag="op", name="warm")
        for _ in range(140):
            nc.tensor.matmul(warm[:, 0:128], ident[:], ident[:],
                             start=True, stop=True, is_transpose=True,
                             skip_group_check=True)
        tbl = big.tile([128, T], bf, tag="tbl")        # cos rows 0:64, sin 64:128
        nc.sync.dma_start(tbl[:], tbl_d)
        tbl2 = big.tile([128, T], bf, tag="tbl2")      # sin rows 0:64, cos 64:128
        nc.sync.dma_start(tbl2[:], tb2_d)
        dma_w("q0", wq_d[0])
        wv = big.tile([128, KB, 256], bf, tag="wv")
        nc.sync.dma_start(wv[:], wv_d)
        dma_w("q1", wq_d[1])
        nc.sync.dma_start(xsb[:, :, 512:1024], xT_d[1])
        wo_sb = big.tile([128, 8, D], bf, tag="wo")

        tblC = tbl      # [cos; cos] both halves
        tblS = tbl2     # [-sin; sin]

        # qk: 8 q blocks (2h+s) then 2 k blocks (8+s); roped, dh-permuted
        qk = big.tile([128, 10, T], bf, tag="qk")
        vsb = big.tile([128, NTB, 257], bf, tag="vsb")
        nc.vector.memset(vsb[:, :, 256:257], 1.0)
        osbT = big.tile([128, 8, T], bf, tag="osbT")   # o transposed, ch-major

        # ---- emission helpers -------------------------------------------
        def proj_group(widx, blk, ts):
            """Q/K projection of one 128-row block for one 512-token span,
            PSUM drained by DVE cast, rope applied by DVE into qk[:, blk]."""
            wt = wts[widx]
            tsl = slice(ts * 512, (ts + 1) * 512)
            ps = pmm.tile([128, 512], f32, tag="mm", name="psp")
            for k in range(KB):
                nc.tensor.matmul(ps[:], wt[:, k], xsb[:, k, tsl],
                                 start=(k == 0), stop=(k == KB - 1))
            # full-width rope: qk = raw*[cos;cos64] + swap(raw)*[-sin;sin0]
            raw = rtmp.tile([128, 512], bf, tag="raw")
            nc.vector.tensor_copy(raw[:], ps[:])
            sw = rtmp.tile([128, 512], bf, tag="sw")
            nc.vector.tensor_copy(sw[0:64, :], raw[64:128, :])
            nc.vector.tensor_copy(sw[64:128, :], raw[0:64, :])
            t1 = rtmp.tile([128, 512], bf, tag="t1")
            t2 = rtmp.tile([128, 512], bf, tag="t2")
            nc.vector.tensor_tensor(t1[:], sw[:], tblS[:, tsl], mult)
            nc.vector.tensor_tensor(t2[:], raw[:], tblC[:, tsl], mult)
            nc.vector.tensor_tensor(qk[:, blk, tsl], t2[:], t1[:], add)

        def v_group(tb):
            ps = pmm.tile([128, 512], f32, tag="mm", name="psv")
            for k in range(KB):
                nc.tensor.matmul(ps[:, 0:256], xsb[:, k, tb * 128:(tb + 1) * 128],
                                 wv[:, k], start=(k == 0), stop=(k == KB - 1))
            nc.vector.tensor_copy(vsb[:, tb, 0:256], ps[:, 0:256])

        def scores_block(h, s, i, J, e_t):
            """One k-block of scores for (head h, stream s, superblock J):
            matmul + exp (ACT) + causal mask for the diagonal block."""
            il = i - 4 * J
            off = il * 128 if il >= 0 else 0
            fd = 512 - off
            ps = pmm.tile([128, 512], f32, tag="mm", name="pssc")
            nc.tensor.matmul(ps[:, 0:fd], qk[:, 8 + s, i * 128:(i + 1) * 128],
                             qk[:, 2 * h + s, J * 512 + off:(J + 1) * 512],
                             start=True, stop=True)
            nc.scalar.activation(e_t[:, i, off:512], ps[:, 0:fd],
                                 mybir.ActivationFunctionType.Exp)
            if il >= 0:
                eng = nc.gpsimd if J == 0 else nc.vector
                eng.tensor_tensor(e_t[:, i, off:off + 128],
                                  e_t[:, i, off:off + 128], tri[:], mult)

        def rsqrt_newton(vb):
            """rstd = 1/sqrt(vb) elementwise on [128,4] via DVE only."""
            y = spool.tile([128, 4], f32, tag="ny")
            t = spool.tile([128, 4], f32, tag="nt")
            nc.vector.tensor_scalar(y[:].bitcast(i32), vb[:].bitcast(i32),
                                    1, None, shr)
            nc.vector.tensor_tensor(y[:].bitcast(i32), magic[:].bitcast(i32),
                                    y[:].bitcast(i32), sub)
            for _ in range(2):
                nc.vector.tensor_tensor(t[:], y[:], y[:], mult)
                nc.vector.tensor_tensor(t[:], t[:], vb[:], mult)
                nc.vector.tensor_scalar(t[:], t[:], -0.5, 1.5, mult, add)
                nc.vector.tensor_tensor(y[:], y[:], t[:], mult)
                if NEWTON_ITERS == 1:
                    break
            return y

        def av_jl(st, jl):
            """AV matmuls + stream combine + accumulated GN stats for one
            128-token query block; frees its PSUM immediately."""
            h, J, e1, e2 = st["h"], st["J"], st["e1"], st["e2"]
            jg = 4 * J + jl
            qq = slice(jl * 128, (jl + 1) * 128)
            u1f = pav.tile([128, 512], f32, tag="u1", name="u1")
            u2f = pav.tile([128, 512], f32, tag="u2", name="u2")
            u1 = u1f[:, 0:257]
            u2 = u2f[:, 0:257]
            for i in range(jg + 1):
                vt = vsb[:, i, :]
                nc.tensor.matmul(u1, e1[:, i, qq], vt,
                                 start=(i == 0), stop=(i == jg),
                                 skip_group_check=True)
                nc.tensor.matmul(u2, e2[:, i, qq], vt,
                                 start=(i == 0), stop=(i == jg),
                                 skip_group_check=True)
            r1 = spool.tile([128, 1], f32, tag="r1")
            r2 = spool.tile([128, 1], f32, tag="r2")
            nc.vector.reciprocal(r1[:], u1[:, 256:257])
            nc.vector.reciprocal(r2[:], u2[:, 256:257])
            nc.vector.tensor_scalar_mul(r2[:], r2[:], -lam)
            o1 = opool.tile([128, 256], bf, tag="o1", name=f"o1_{jl}")
            nc.vector.tensor_scalar_mul(o1[:], u1f[:, 0:256], r1[:])
            nc.vector.scalar_tensor_tensor(o1[:], u2f[:, 0:256], r2[:],
                                           o1[:], mult, add)
            st6 = spool.tile([128, 6], f32, tag="st6")
            mv = spool.tile([128, 2], f32, tag="mv", name=f"mv_{jl}")
            nc.vector.bn_stats(st6[:], o1[:])
            nc.vector.bn_aggr(mv[:], st6[:])
            st["o1s"].append(o1)
            st["mvs"].append(mv)

        def av_fin(st, fillers=()):
            """GroupNorm normalize + transposes into osbT. `fillers` are
            tensor-work closures emitted while the DVE chain completes."""
            h, J = st["h"], st["J"]
            vb = spool.tile([128, 4], f32, tag="vb", name="vb")
            for jl in range(4):
                nc.vector.tensor_scalar_add(vb[:, jl:jl + 1],
                                            st["mvs"][jl][:, 1:2], GN_EPS)
            rst = rsqrt_newton(vb)
            o2s = []
            for jl in range(4):
                o2 = opool.tile([128, 256], bf, tag="o2", name=f"o2_{jl}")
                nc.vector.tensor_scalar(o2[:], st["o1s"][jl][:],
                                        st["mvs"][jl][:, 0:1],
                                        rst[:, jl:jl + 1], sub, mult)
                o2s.append(o2)
            for f in fillers:
                f()
            # transposes (q,ch)->(ch,q) into resident osbT, via bf16 psum
            pst1 = pav.tile([128, 512], bf, tag="u1", name="pst1")
            pst2 = pav.tile([128, 512], bf, tag="u2", name="pst2")
            for jl in range(4):
                cc = slice(jl * 128, (jl + 1) * 128)
                nc.tensor.matmul(pst1[:, cc], o2s[jl][:, 0:128], ident[:],
                                 start=True, stop=True, is_transpose=True,
                                 skip_group_check=True)
                nc.tensor.matmul(pst2[:, cc], o2s[jl][:, 128:256], ident[:],
                                 start=True, stop=True, is_transpose=True,
                                 skip_group_check=True)
            jsl = slice(J * 512, (J + 1) * 512)
            nc.vector.tensor_copy(osbT[:, 2 * h, jsl], pst1[:])
            nc.vector.tensor_copy(osbT[:, 2 * h + 1, jsl], pst2[:])

        def av_state(h, J, e1, e2):
            return {"h": h, "J": J, "e1": e1, "e2": e2,
                    "o1s": [], "mvs": []}

        # o-projection: per token block tb, 4 psum chunks of 512 out-cols,
        # each an 8-step accumulation; emitted as 4-mm half-chunks.
        # lo=0 halves touch only cb 0..3 (heads 0,1); lo=4 touch cb 4..7.
        ostage = {}
        odrained = {}

        def oproj_half(tb, n, lo, split_dma=False):
            if tb not in ostage:
                ostage[tb] = stage.tile([128, D], bf, tag="so",
                                        name=f"so{tb}")
                odrained[tb] = 0
            if lo == 0:
                ostage[(tb, n)] = pop.tile([128, 512], f32, tag="op",
                                           name=f"pso{tb}_{n}")
            pso = ostage[(tb, n)]
            for cb in range(lo, lo + 4):
                nc.tensor.matmul(pso[:], osbT[:, cb, tb * 128:(tb + 1) * 128],
                                 wo_sb[:, cb, n * 512:(n + 1) * 512],
                                 start=(cb == 0), stop=(cb == 7),
                                 skip_group_check=True)
            if lo == 4:
                nc.vector.tensor_copy(ostage[tb][:, n * 512:(n + 1) * 512],
                                      pso[:])
                del ostage[(tb, n)]
                odrained[tb] += 1
                if split_dma:
                    eng = nc.scalar if tb % 2 else nc.sync
                    eng.dma_start(
                        out_d[tb * 128:(tb + 1) * 128, n * 512:(n + 1) * 512],
                        ostage[tb][:, n * 512:(n + 1) * 512])
                    if odrained[tb] == 4:
                        del ostage[tb]
                elif odrained[tb] == 4:
                    nc.sync.dma_start(out_d[tb * 128:(tb + 1) * 128, :],
                                      ostage[tb][:])
                    del ostage[tb]

        # ---- block AB: projections + J0 attention interleaved ------------
        proj_group("k0", 8, 0)
        proj_group("k1", 9, 0)
        proj_group("q0", 0, 0)
        proj_group("q1", 1, 0)
        v_group(0)
        proj_group("k0", 8, 1)
        proj_group("k1", 9, 1)
        dma_w("q2", wq_d[2])
        dma_w("q3", wq_d[3])
        proj_group("q0", 0, 1)
        proj_group("q1", 1, 1)
        dma_w("q4", wq_d[4])
        dma_w("q5", wq_d[5])
        for half in range(2):
            nc.sync.dma_start(wo_sb[:, 4 * half:4 * half + 4],
                              wo_d[:, 4 * half:4 * half + 4])
        for tb in range(1, 4):
            v_group(tb)

        # per-h filler slots (one after each of the 8 J0 scores blocks);
        # av-jl chunks of the previous head fill the remaining slots
        fillers = {
            0: [("q2", 2, 0), ("q2", 2, 1), ("q3", 3, 0), ("q3", 3, 1),
                ("q4", 4, 0), ("q4", 4, 1), ("q5", 5, 0), ("q5", 5, 1)],
            1: [("q6", 6, 0), ("q6", 6, 1), ("q7", 7, 0), ("q7", 7, 1)],
            2: [("v", 4), ("v", 5)],
            3: [("v", 6), ("v", 7)],
        }
        dma_after = {0: ["q6", "q7"], 1: [], 2: [], 3: []}

        def emit_filler(f):
            if f[0] == "v":
                v_group(f[1])
            else:
                proj_group(*f)

        prev_st = None
        for h in range(HPC):
            e1 = epool.tile([128, 8, 512], bf, tag="e1", name=f"e1_{h}j0")
            e2 = epool.tile([128, 8, 512], bf, tag="e2", name=f"e2_{h}j0")
            st = av_state(h, 0, e1, e2)
            blocks = [(s, i) for s in range(2) for i in range(4)]
            fl = list(fillers[h])
            # hold the last filler back: it runs inside av_fin while the
            # GroupNorm DVE chain completes, so the transposes don't stall
            # the tensor queue (the held item must not gate J0 scores)
            fin_fill = fl.pop() if (prev_st is not None and fl) else None
            for bi, (s, i) in enumerate(blocks):
                scores_block(h, s, i, 0, e1 if s == 0 else e2)
                # av-jl chunks of the previous head early (frees PSUM, puts
                # the combine ahead of the rope work on the DVE queue)
                if prev_st is not None and bi % 2 == 0:
                    av_jl(prev_st, bi // 2)
                elif fl:
                    f = fl.pop(0)
                    if f is not None:
                        emit_filler(f)
                if prev_st is not None and fl and bi % 2 == 1:
                    f = fl.pop(0)
                    if f is not None:
                        emit_filler(f)
            for widx in dma_after[h]:
                dma_w(widx, wq_d[int(widx[1:])])
            if prev_st is not None:
                ff = fin_fill
                av_fin(prev_st,
                       fillers=[] if ff is None
                       else [lambda: emit_filler(ff)])
            prev_st = st

        # ---- block C: J1 attention + o-projection interleaved ------------
        # J0 o-proj halves: lo=0 (cb 0..3) are legal once heads 0,1 J0 are
        # transposed; lo=4 (cb 4..7) only after the J0 pipeline drains (end
        # of h=0 iteration here).
        opq = [(tb, n, lo) for tb in range(4) for n in range(4)
               for lo in (0, 4)]
        reserve = [opq.pop() for _ in range(6)][::-1]   # tail halves for av_fin(3,J1)

        for h in range(HPC):
            e1 = epool.tile([128, 8, 512], bf, tag="e1", name=f"e1_{h}j1")
            e2 = epool.tile([128, 8, 512], bf, tag="e2", name=f"e2_{h}j1")
            st = av_state(h, 1, e1, e2)
            blocks = [(s, i) for s in range(2) for i in range(8)]
            ps_prev = prev_st
            spacers = [lambda jl=jl: av_jl(ps_prev, jl) for jl in range(4)]
            if h == 0:
                # prev = (3, J0): lo=0 halves (cb 0..3) are legal now; lo=4
                # (cb 4..7) only after its av_fin lands the J0 transposes,
                # so those go in the post-scores tail
                pre = [opq[0], opq[2]]                  # (t0n0lo0) (t0n1lo0)
                del opq[2], opq[0]
                tail, opq[:3] = opq[:3], []             # lo4s + next lo0
                spacers += [lambda a=a: oproj_half(*a) for a in pre]
                spacers.append(lambda: av_fin(ps_prev))
                spacers += [lambda a=a: oproj_half(*a) for a in tail]
            else:
                nh = 4 if h < 3 else 2
                pre, opq[:nh] = opq[:nh], []
                post, opq[:2] = opq[:2], []
                spacers += [lambda a=a: oproj_half(*a) for a in pre]
                spacers.append(lambda: av_fin(
                    ps_prev,
                    fillers=[lambda a=a: oproj_half(*a) for a in post]))
            for bi, (s, i) in enumerate(blocks):
                scores_block(h, s, i, 1, e1 if s == 0 else e2)
                if bi % 2 == 1 and spacers:
                    f = spacers.pop(0)
                    f()
            while spacers:
                spacers.pop(0)()
            prev_st = st

        res_ops = [lambda a=a: oproj_half(*a) for a in reserve]
        for jl in range(4):
            av_jl(prev_st, jl)
            res_ops.pop(0)()
        av_fin(prev_st, fillers=res_ops)

        # ---- block D: J1 o-projection ------------------------------------
        for a in opq:
            oproj_half(*a)
        for tb in range(4, 8):
            for n in range(4):
                oproj_half(tb, n, 0)
                oproj_half(tb, n, 4, split_dma=(tb >= 6))


# ------------------------- host side  ------------------------------------

_ROPE_PERM = np.concatenate([np.arange(0, DH, 2), np.arange(1, DH, 2)])


def _prep(x, Wq, Wk, Wv, Wo, lambda_q1, lambda_k1, lambda_q2, lambda_k2,
          gn_weight, gn_bias, pos):
    lam = float(np.exp(np.sum(lambda_q1 * lambda_k1))
                - np.exp(np.sum(lambda_q2 * lambda_k2)) + LAMBDA_INIT)
    scale = DH ** -0.5

    posf = pos.astype(np.float64)
    inv = 1.0 / (ROPE_BASE ** (np.arange(0, DH, 2, dtype=np.float32) / DH))
    freqs = (posf[:, None] * inv[None, :].astype(np.float64)).astype(np.float32)
    cosv = np.cos(freqs).T          # (64, T)
    sinv = np.sin(freqs).T
    tbl = np.ascontiguousarray(
        np.concatenate([cosv, cosv], axis=0).astype(bf16))      # (128, T)
    tbl2 = np.ascontiguousarray(
        np.concatenate([-sinv, sinv], axis=0).astype(bf16))     # (128, T)

    # Wq: (H,2,DH,D), rope-permute DH, fold score scale
    Wq4 = (Wq.reshape(H, 2, DH, D)[:, :, _ROPE_PERM, :] * scale).astype(np.float32)
    Wk4 = Wk.reshape(KV, 2, DH, D)[:, :, _ROPE_PERM, :].astype(np.float32)
    Wv3 = Wv.reshape(KV, 2 * DH, D).astype(np.float32)

    s1 = 1.0 - LAMBDA_INIT
    Wo_f = (Wo * (gn_weight * s1)[None, :]).astype(np.float32)   # (D, 4096)
    bias_out = (gn_bias * s1).astype(np.float32) @ Wo.T.astype(np.float32)

    def to_sb(w2d, cols):           # (D, cols) -> (128, KB, cols) bf16
        return np.ascontiguousarray(
            w2d.reshape(KB, 128, cols).transpose(1, 0, 2)).astype(bf16)

    in_maps = []
    for c in range(NCORES):
        b, g = c // 4, c % 4
        # x for batch b: (T, D) -> (D, T) -> (NJ, 128, KB, 512)
        xT = x[b].T.astype(np.float32)
        x3 = xT.reshape(KB, 128, NJ, 512).transpose(2, 1, 0, 3)
        xT3 = np.ascontiguousarray(x3).astype(bf16)

        # q heads 4g..4g+3: block 2hh+s = Wq4[4g+hh, s] (DH rows)
        wqT = Wq4[4 * g:4 * g + 4].reshape(8 * DH, D).T       # (D, 1024)
        wkT = Wk4[g].reshape(2 * DH, D).T                     # (D, 256)
        wvT = Wv3[g].T                                        # (D, 256)
        woT = Wo_f[:, 1024 * g:1024 * (g + 1)].T              # (1024 ch, D)
        wo3 = np.ascontiguousarray(
            woT.reshape(8, 128, D).transpose(1, 0, 2)).astype(bf16)
        wq_stack = np.stack([to_sb(wqT[:, cb * 128:(cb + 1) * 128], 128)
                             for cb in range(8)])             # (8,128,KB,128)
        wk_stack = np.stack([to_sb(wkT[:, cb * 128:(cb + 1) * 128], 128)
                             for cb in range(2)])             # (2,128,KB,128)
        in_maps.append({
            "xT": xT3,
            "wqT": wq_stack,
            "wkT": wk_stack,
            "wvT": to_sb(wvT, 256),
            "woT": wo3,
            "tbl": tbl,
            "tbl2": tbl2,
        })
    return lam, in_maps, bias_out


LAST_RESULT = None


def kernel(**inputs):
    global LAST_RESULT
    inputs = {k: np.asarray(v) for k, v in inputs.items()}
    lam, in_maps, bias_out = _prep(**inputs)
    nc = build_program(lam)
    res = run_bass_kernel_spmd(nc, in_maps, core_ids=list(range(NCORES)))
    LAST_RESULT = res
    out = np.zeros((B, T, D), np.float32)
    for c in range(NCORES):
        out[c // 4] += res.results[c]["out"].astype(np.float32)
    out += bias_out[None, None, :]
    return out.astype(np.float32)


if __name__ == "__main__":
    import reference
    ins = {k: np.asarray(v) for k, v in reference.setup_inputs().items()}
    got = kernel(**ins)
    exp = np.asarray(reference.reference(**ins))
    rel = np.linalg.norm(got - exp) / np.linalg.norm(exp)
    print("rel err:", rel)
